# revision 29
# baseline (speedup 1.0000x reference)
"""Causal self-attention (GQA + RMS-norm + RoPE) Trainium2 Bass kernel.

Sharding over 8 NeuronCores: 2-way data parallel (batch) x 4-way head
parallel (one GQA group of 4 q-heads + 1 kv-head per core).  Each core
computes q/k/v projections for its group, flash-style causal attention
(scores kept transposed [k, q] so softmax sums ride the tensor engine),
and a partial output projection.  Host sums the 4 per-group partials per
batch.

All matmul operands are bf16 (fp32 accumulation in PSUM); softmax /
norm statistics are fp32.  RMS-norm bounds |scores| <= gain*sqrt(hd), so
softmax needs no max-subtraction.

v3 over the original baseline:
- x is loaded j-block-major (4 big DMAs) so kv projections start early;
  weights load as single DMAs.
- rms-norm stats: per 4-chain group, the four [1,512] mean-squares are
  staged into one [1,2048] row, one ACT Sqrt + one fast-approx DVE
  reciprocal (keeps Sqrt<->Exp ACT table swaps off the attention phase
  and replaces the 3.3us DVE reciprocals).
- rope/stage tiles are bf16 (2x/4x DVE modes); gain and 1/sqrt(hd) fold
  into the Sqrt scale/bias APs.
- attention denominators use the fast-approx reciprocal too.
- output partials are written bf16 (halves the output DMA).
"""

import numpy as np
import ml_dtypes

import concourse.bacc as bacc
import concourse.mybir as mybir
from concourse.tile import TileContext
from concourse.bass_utils import run_bass_kernel_spmd

BF16 = mybir.dt.bfloat16
F32 = mybir.dt.float32
F32R = mybir.dt.float32r
AF = mybir.ActivationFunctionType
bf = ml_dtypes.bfloat16

B, S, D = 2, 2048, 2048
H, HKV, HD = 16, 4, 128
RQ = H // HKV            # q heads per kv group (4)
NCORES = 8
NDT = D // 128           # 16 contraction tiles
NST = S // 512           # 4 query/sequence 512-tiles
NKT = S // 128           # 16 key 128-tiles
EPS = float(np.finfo(np.float32).eps)

_PROG_CACHE = {}


def _build_program(n_timing_iters=1):
    nc = bacc.Bacc("TRN2", debug=False, enable_asserts=False, num_devices=NCORES)

    xT_d = nc.dram_tensor("xT", [NST, 128, NDT, 512], BF16, kind="ExternalInput")
    wqT_d = nc.dram_tensor("wqT", [128, NDT, 512], BF16, kind="ExternalInput")
    wkT_d = nc.dram_tensor("wkT", [128, NDT, 128], BF16, kind="ExternalInput")
    wvT_d = nc.dram_tensor("wvT", [128, NDT, 128], BF16, kind="ExternalInput")
    wpT_d = nc.dram_tensor("wpT", [128, NDT * RQ, 128], BF16, kind="ExternalInput")
    cosF_d = nc.dram_tensor("cosF", [128, S], BF16, kind="ExternalInput")
    sinF_d = nc.dram_tensor("sinF", [128, S], BF16, kind="ExternalInput")
    cfs_d = nc.dram_tensor("cfs", [1, 16], F32, kind="ExternalInput")
    onescol_b_d = nc.dram_tensor("onescol_b", [128, 1], BF16, kind="ExternalInput")
    onesrow_d = nc.dram_tensor("onesrow", [1, 128], F32R, kind="ExternalInput")
    idtr_d = nc.dram_tensor("idtr", [128, 256], BF16, kind="ExternalInput")
    outT_d = nc.dram_tensor("outT", [NDT, 128, S], BF16, kind="ExternalOutput")
    rk_d = nc.dram_tensor("rk_scratch", [1, S], F32, kind="Internal")

    with TileContext(nc) as tc:
        with tc.tile_pool(name="res", bufs=1) as res, \
             tc.tile_pool(name="work", bufs=2) as wk, \
             tc.tile_pool(name="pwork", bufs=2, space="PSUM") as pw:

            # ---- resident tiles (allocated once) ----
            xT = res.tile([128, NST * NDT, 512], BF16)     # [d-part, (j, dt), s]
            wqT = res.tile([128, NDT, 512], BF16)
            wkT = res.tile([128, NDT, 128], BF16)
            wvT = res.tile([128, NDT, 128], BF16)
            wpT = res.tile([128, NDT * RQ, 128], BF16)
            cosF = res.tile([128, S], BF16)
            sinF = res.tile([128, S], BF16)
            cfs = res.tile([1, 16], F32)
            onescol_b = res.tile([128, 1], BF16)
            onesrow = res.tile([1, 128], F32R)
            idtr = res.tile([128, 256], BF16)
            kT = res.tile([128, S], BF16)
            qT = [res.tile([128, S], BF16, name=f"qT{h}", tag=f"qT{h}")
                  for h in range(RQ)]
            yT = res.tile([128, RQ, S], BF16)
            rkT = res.tile([128, NKT], F32)                # 1/rms_k per key
            vTst = res.tile([128, S], BF16)                # v^T staging
            V_all = res.tile([128, S], BF16)               # v natural, kt-major

            ident = idtr[:, 0:128]
            triu = idtr[:, 128:256]

            def body(_iv=None):
                # ---- load residents ----
                # k/v weights + consts first (first chains need them);
                # cos/sin/idtr/wq deferred past the x blocks that gate the PE
                nc.sync.dma_start(wkT[:], wkT_d[:])
                nc.sync.dma_start(wvT[:], wvT_d[:])
                nc.sync.dma_start(cfs[:], cfs_d[:])
                nc.sync.dma_start(onescol_b[:], onescol_b_d[:])
                nc.sync.dma_start(onesrow[:], onesrow_d[:])
                for j in range(NST):
                    for hb in range(2):
                        nc.sync.dma_start(
                            xT[:, j * NDT + 8 * hb:j * NDT + 8 * (hb + 1), :],
                            xT_d[j][:, 8 * hb:8 * (hb + 1), :])
                    if j == 0:
                        nc.sync.dma_start(cosF[:], cosF_d[:])
                        nc.sync.dma_start(sinF[:], sinF_d[:])
                    elif j == 1:
                        nc.sync.dma_start(wqT[:], wqT_d[:])
                nc.sync.dma_start(idtr[:], idtr_d[:])
                nc.sync.dma_start(wpT[:], wpT_d[:])

                def proj_accum(wt_all, m0, m1, j, tag="big", tbufs=3):
                    """psum [128,512] = sum_d W[d].T @ xT[d, s-slice]"""
                    acc = pw.tile([128, 512], F32, name="acc", tag=tag,
                                  bufs=tbufs)
                    for dt in range(NDT):
                        lhsT = wt_all[:, dt, m0:m1]
                        rhs = xT[:, j * NDT + dt, :]
                        nc.tensor.matmul(acc[:], lhsT, rhs,
                                         start=(dt == 0), stop=(dt == NDT - 1))
                    return acc

                def chain_front(acc, j, rope, stage_dve=False):
                    """stage/swap/square/ms for one 512-wide block; rope[j]
                    holds (stage*cos + swap*sin) bf16; returns the ms psum."""
                    stage = wk.tile([128, 512], BF16, name="stage", tag="stage",
                                    bufs=3)
                    if stage_dve:
                        nc.vector.tensor_copy(stage[:], acc[:])
                    else:
                        nc.scalar.copy(stage[:], acc[:])
                    swap = wk.tile([128, 512], BF16, name="swap", tag="swap",
                                   bufs=3)
                    nc.gpsimd.dma_start(swap[0:64, :], stage[64:128, :])
                    nc.gpsimd.dma_start(swap[64:128, :], stage[0:64, :])
                    sq = wk.tile([128, 512], BF16, name="sq", tag="sq", bufs=2)
                    nc.vector.tensor_mul(sq[:], stage[:], stage[:])
                    ms = pw.tile([1, 512], F32, name="ms", tag="qp", bufs=1)
                    nc.tensor.matmul(ms[:], onescol_b[:], sq[:],
                                     start=True, stop=True)
                    sl = slice(512 * j, 512 * j + 512)
                    nc.vector.tensor_mul(stage[:], stage[:], cosF[:, sl])
                    nc.vector.tensor_mul(swap[:], swap[:], sinF[:, sl])
                    nc.vector.tensor_add(rope[:], stage[:], swap[:])
                    return ms

                def norm_batch_tail(entries, sc_i, bi_i, dest):
                    """entries: list of (j, rope, msb, rrb) rows staged at
                    [0:1, 512j:+512]; one Sqrt + one approx-reciprocal for the
                    whole group, then per-j broadcast + final scale."""
                    msb = entries[0][2]
                    rrb = entries[0][3]
                    lo = min(512 * j for j, _, _, _ in entries)
                    hi = max(512 * j + 512 for j, _, _, _ in entries)
                    nc.scalar.activation(rrb[0:1, lo:hi], msb[0:1, lo:hi],
                                         AF.Sqrt,
                                         bias=cfs[0:1, bi_i:bi_i + 1],
                                         scale=cfs[0:1, sc_i:sc_i + 1])
                    nc.vector.reciprocal_approx_fast(msb[0:1, lo:hi],
                                                     rrb[0:1, lo:hi])
                    for j, rope, _, _ in entries:
                        sl = slice(512 * j, 512 * j + 512)
                        Rb = wk.tile([128, 512], F32, name="Rb", tag="Rb",
                                     bufs=3)
                        nc.gpsimd.partition_broadcast(Rb[:], msb[0:1, sl])
                        nc.vector.tensor_mul(dest[:, sl], rope[:], Rb[:])

                def norm_group():
                    """Shared staging rows for one 4-chain norm group."""
                    msb = wk.tile([1, S], F32, name="msb", tag="msb", bufs=1)
                    rrb = wk.tile([1, S], F32, name="rrb", tag="rrb", bufs=1)
                    return {"msb": msb, "rrb": rrb, "entries": []}

                def norm_front(grp, kind, h, j, tag, tbufs):
                    if kind == "k":
                        acc = proj_accum(wkT, 0, 128, j, tag, tbufs)
                        rope = kT[:, 512 * j:512 * j + 512]
                    else:
                        acc = proj_accum(wqT, 128 * h, 128 * h + 128, j,
                                         tag, tbufs)
                        rope = wk.tile([128, 512], BF16, name="rope",
                                       tag="rope", bufs=4)
                    ms = chain_front(acc, j, rope, stage_dve=(kind == "q" and h > 0))
                    nc.scalar.copy(grp["msb"][0:1, 512 * j:512 * j + 512],
                                   ms[:])
                    if kind != "k":
                        grp["entries"].append((j, rope, grp["msb"],
                                               grp["rrb"]))

                def k_norm_tail(grp):
                    """1/rms_k lands as a per-key column via a DRAM
                    round-trip; applied later as the exp per-partition
                    scale (free affine)."""
                    msb, rrb = grp["msb"], grp["rrb"]
                    nc.scalar.activation(rrb[0:1, :], msb[0:1, :], AF.Sqrt,
                                         bias=cfs[0:1, 1:2],
                                         scale=cfs[0:1, 0:1])
                    nc.vector.reciprocal_approx_fast(msb[0:1, :], rrb[0:1, :])
                    nc.sync.dma_start(rk_d[:], msb[0:1, :])
                    nc.sync.dma_start(
                        rkT[:],
                        rk_d[:].rearrange("a (kt p) -> (a p) kt", p=128))

                def v_job(j, tag, tbufs):
                    acc = proj_accum(wvT, 0, 128, j, tag, tbufs)
                    nc.scalar.copy(vTst[:, 512 * j:512 * j + 512], acc[:])

                # ---- kv + q0 prologue: k/v interleaved with the arriving x
                # blocks; v(j2/j3)+transposes after q0 so the PE covers q0's
                # norm tail ----
                kgrp = norm_group()
                for j in range(NST):
                    norm_front(kgrp, "k", 0, j, "big", 3)
                    v_job(j, "acc", 2)
                k_norm_tail(kgrp)
                q0 = norm_group()
                for j in range(NST):
                    norm_front(q0, "q", 0, j, "small", 2)
                norm_batch_tail(q0["entries"], 2, 6, qT[0])

                # ---- v^T -> V transposes (PE; also cover q0's tail) ----
                for kt in range(NKT):
                    tp = pw.tile([128, 128], BF16, name="tp", tag="acc", bufs=2)
                    nc.tensor.transpose(tp[:], vTst[:, 128 * kt:128 * kt + 128],
                                        ident)
                    nc.scalar.copy(V_all[:, 128 * kt:128 * kt + 128],
                                   tp[:])

                def attention_block(h, j):
                    """Causal attention for queries [512j, 512j+512), head h.
                    d/PV matmuls trail the S-matmul/exp pipeline by 2 k-tiles
                    so PE never waits on ACT."""
                    nkt = 4 * j + 4
                    n_dmm = 2 * j + 4        # pairs + last-full + 3 diagonal
                    acc_y = pw.tile([128, 512], F32, name="acc_y", tag="acc",
                                    bufs=2)
                    acc_d = pw.tile([1, 512], F32, name="acc_d", tag="small",
                                    bufs=2)
                    lagged = []
                    pend_full = []
                    dcnt = [0]

                    def dmm(P_, c0_):
                        dcnt[0] += 1
                        nc.tensor.matmul(acc_d[0:1, c0_:512], onescol_b[:],
                                         P_[:, c0_:512],
                                         start=(dcnt[0] == 1),
                                         stop=(dcnt[0] == n_dmm),
                                         skip_group_check=True)

                    def consume(kt, P, c0):
                        # denominator: sum full-tile pairs on DVE first so the
                        # ones-matmul streams each pair once
                        if c0 == 0:
                            pend_full.append(P)
                            if len(pend_full) == 2:
                                Pa, Pb = pend_full
                                pend_full.clear()
                                Ps = wk.tile([128, 512], BF16, name="Ps",
                                             tag="Ps", bufs=2)
                                nc.vector.tensor_add(Ps[:], Pa[:], Pb[:])
                                dmm(Ps, 0)
                            elif kt == 4 * j:
                                dmm(pend_full.pop(), 0)
                        else:
                            dmm(P, c0)
                        nc.tensor.matmul(acc_y[:, c0:512],
                                         V_all[:, 128 * kt:128 * kt + 128],
                                         P[:, c0:512],
                                         start=(kt == 0), stop=(kt == nkt - 1),
                                         skip_group_check=True)

                    for kt in range(nkt):
                        c0 = max(0, 128 * (kt - 4 * j))
                        ps = pw.tile([128, 512], F32, name="ps", tag="big",
                                     bufs=3)
                        nc.tensor.matmul(
                            ps[:, c0:512],
                            kT[:, 128 * kt:128 * kt + 128],
                            qT[h][:, 512 * j + c0:512 * j + 512],
                            start=True, stop=True)
                        P = wk.tile([128, 512], BF16, name="P", tag="P", bufs=4)
                        nc.scalar.activation(P[:, c0:512], ps[:, c0:512], AF.Exp,
                                             scale=rkT[:, kt:kt + 1])
                        if kt >= 4 * j:
                            nc.vector.tensor_mul(P[:, c0:c0 + 128],
                                                 P[:, c0:c0 + 128], triu)
                        lagged.append((kt, P, c0))
                        if len(lagged) > 2:
                            consume(*lagged.pop(0))
                    while lagged:
                        consume(*lagged.pop(0))
                    rd = wk.tile([1, 512], F32, name="rd", tag="rd", bufs=1)
                    nc.vector.reciprocal_approx_fast(rd[:], acc_d[:])
                    Rd = wk.tile([128, 512], F32, name="Rd", tag="Rd",
                                 bufs=2)
                    nc.gpsimd.partition_broadcast(Rd[:], rd[0:1, :])
                    nc.vector.tensor_mul(yT[:, h, 512 * j:512 * j + 512],
                                         acc_y[:], Rd[:])

                def out_proj_chunk(sjj):
                    """out^T[:, 512sjj:+512] for all dt (one s-chunk), streamed
                    out per-dt.  Runs interleaved with attention(h3)."""
                    for dt in range(NDT):
                        po = pw.tile([128, 512], F32, name="po", tag="acc",
                                     bufs=2)
                        for h in range(RQ):
                            nc.tensor.matmul(
                                po[:], wpT[:, dt * RQ + h, :],
                                yT[:, h, 512 * sjj:512 * sjj + 512],
                                start=(h == 0), stop=(h == RQ - 1))
                        osb = wk.tile([128, 512], BF16, name="osb", tag="osb",
                                      bufs=3)
                        if dt % 2 == 0:
                            nc.vector.tensor_copy(osb[:], po[:])
                        else:
                            nc.scalar.copy(osb[:], po[:])
                        nc.sync.dma_start(
                            outT_d[dt][:, 512 * sjj:512 * sjj + 512], osb[:])

                # ---- per q-head: q(h+1) chain fronts interleaved per-block
                # with attention(h) (attention block j only needs qT block j),
                # batch tail hidden under the last (ACT-heavy) block; the
                # output projection chunks ride along with attention(h3) ----
                for h in range(RQ):
                    grp = norm_group() if h + 1 < RQ else None
                    for j in range(NST):
                        if grp is not None:
                            norm_front(grp, "q", h + 1, j, "qp", 1)
                            if j == NST - 1:
                                norm_batch_tail(grp["entries"], 2 + h + 1,
                                                6 + h + 1, qT[h + 1])
                        attention_block(h, j)
                        if h == RQ - 1:
                            out_proj_chunk(j)

            if n_timing_iters > 1:
                with tc.For_i(0, n_timing_iters, 1):
                    body()
            else:
                body()

    nc.compile()
    return nc


def _get_program(n_timing_iters=1):
    key = n_timing_iters
    if key not in _PROG_CACHE:
        _PROG_CACHE[key] = _build_program(n_timing_iters)
    return _PROG_CACHE[key]


def _host_inputs(x, Wq, Wk, Wv, Wproj, q_gain):
    """Build the 8 per-core input maps (host-side layout prep)."""
    inv = 1.0 / (10000.0 ** (np.arange(0, HD, 2, dtype=np.float64) / HD))
    t = np.arange(S, dtype=np.float64)
    fr = np.outer(t, inv).astype(np.float32)          # [S, 64]
    cos = np.cos(fr).astype(np.float32)
    sin = np.sin(fr).astype(np.float32)
    cosF = np.concatenate([cos.T, cos.T], 0).astype(bf)          # [128, S]
    sinF = np.concatenate([sin.T, -sin.T], 0).astype(bf)

    onescol_b = np.ones((128, 1), bf)
    onesrow = np.ones((1, 128), np.float32)
    ident = np.eye(128, dtype=np.float32)
    triu = (np.arange(128)[None, :] >= np.arange(128)[:, None]).astype(np.float32)
    idtr = np.concatenate([ident, triu], 1).astype(bf)

    # x: [j, p, dt, c] = x[b, 512j+c, 128dt+p]
    xT = [np.ascontiguousarray(
            x[b].reshape(NST, 512, NDT, 128).transpose(0, 3, 2, 1)).astype(bf)
          for b in range(B)]

    in_maps = []
    for c in range(NCORES):
        b, g = c // HKV, c % HKV
        wq = np.ascontiguousarray(
            Wq[512 * g:512 * (g + 1)].reshape(512, NDT, 128)
            .transpose(2, 1, 0)).astype(bf)
        wk_ = np.ascontiguousarray(
            Wk[128 * g:128 * (g + 1)].reshape(128, NDT, 128)
            .transpose(2, 1, 0)).astype(bf)
        wv = np.ascontiguousarray(
            Wv[128 * g:128 * (g + 1)].reshape(128, NDT, 128)
            .transpose(2, 1, 0)).astype(bf)
        wp = np.ascontiguousarray(
            Wproj[:, 512 * g:512 * (g + 1)]
            .reshape(NDT, 128, RQ, 128).transpose(3, 0, 2, 1)
            .reshape(128, NDT * RQ, 128)).astype(bf)
        g2 = (q_gain[RQ * g: RQ * (g + 1)].astype(np.float64)) ** 2
        cfsv = np.zeros((1, 16), np.float32)
        cfsv[0, 0] = 1.0 / HD                    # k scale
        cfsv[0, 1] = EPS                         # k bias
        cfsv[0, 2:6] = 1.0 / g2                  # q scales
        cfsv[0, 6:10] = EPS * HD / g2            # q biases
        in_maps.append({
            "xT": xT[b],
            "wqT": wq, "wkT": wk_, "wvT": wv, "wpT": wp,
            "cosF": cosF, "sinF": sinF, "cfs": cfsv,
            "onescol_b": onescol_b, "onesrow": onesrow, "idtr": idtr,
        })
    return in_maps


def kernel(x, Wq, Wk, Wv, Wproj, q_gain, _n_timing_iters=1, _return_raw=False,
           **_run_kwargs):
    in_maps = _host_inputs(np.asarray(x, np.float32),
                           np.asarray(Wq, np.float32),
                           np.asarray(Wk, np.float32),
                           np.asarray(Wv, np.float32),
                           np.asarray(Wproj, np.float32),
                           np.asarray(q_gain, np.float32))
    nc = _get_program(_n_timing_iters)
    res = run_bass_kernel_spmd(nc, in_maps, core_ids=list(range(NCORES)),
                               **_run_kwargs)
    if _return_raw:
        return res
    out = np.zeros((B, S, D), np.float32)
    for c in range(NCORES):
        b = c // HKV
        outT = res.results[c]["outT"].astype(np.float32).reshape(D, S)
        out[b] += outT.T
    return out


if __name__ == "__main__":
    rng = np.random.default_rng(0)
    x = rng.standard_normal((B, S, D)).astype(np.float32)
    Wq = (rng.standard_normal((D, D)) * 0.02).astype(np.float32)
    Wk = (rng.standard_normal((512, D)) * 0.02).astype(np.float32)
    Wv = (rng.standard_normal((512, D)) * 0.02).astype(np.float32)
    Wp = (rng.standard_normal((D, D)) * 0.02).astype(np.float32)
    g = np.ones(H, np.float32)
    out = kernel(x, Wq, Wk, Wv, Wp, g)
    print("out", out.shape, out.dtype, float(np.abs(out).max()))
